# revision 6
# baseline (speedup 1.0000x reference)
"""Sliding-tile attention (STA) for nn_Attention_63548336111668 on 8 TRN2 cores.

Strategy:
- tile()/untile() and the per-head sliding-tile-window gather are pure
  permutations/gathers -> done host-side in numpy.
- Work is (head, query-tile) pairs: 216 qtiles x 4 heads. Each core gets a
  contiguous range of 27 qtiles for ALL 4 heads -> every core runs the exact
  same instruction stream (SPMD) on its own shard.
- Per (h, qt): S^T[k,q] = K_d^T.T @ Q_d^T via TensorE (contraction d=64,
  row-packed in pairs via tile_position so two K=64 matmuls share the
  384-cycle stream), P^T = exp(S^T * 1/8) on ScalarE (scores are O(1), no
  max-subtraction needed), O^T[d,q] += V[k,d].T @ P^T accumulated in PSUM
  over key blocks. V carries an appended ones-column, so O^T row 64 = the
  softmax denominator.
- Host divides numerator rows by the denominator row, then un-permutes.

Compute dtype bf16 (inputs converted host-side), accumulation f32.
"""

import sys

sys.path.insert(0, "/opt/trn_rl_repo")

import numpy as np
import ml_dtypes

import concourse.bass as bass
import concourse.tile as tile
from concourse import mybir
from concourse.bass_utils import run_bass_kernel_spmd

# ---------------------------------------------------------------- geometry
NT, NH, NW = 6, 6, 6
TT, TH, TW = 6, 8, 8
TILE = TT * TH * TW            # 384
NTILES = NT * NH * NW          # 216
SEQ = NTILES * TILE            # 82944
WINDOWS = ((2, 1, 1), (1, 2, 2), (1, 1, 2), (1, 1, 1))
HEADS = 4
D = 64
N_CORES = 8
QT_PER_CORE = NTILES // N_CORES  # 27
SCALE = 1.0 / 8.0              # 1/sqrt(64)
BF16 = ml_dtypes.bfloat16

WS = [w[0] * w[1] * w[2] for w in WINDOWS]   # [2, 4, 2, 1]

# Schraudolph-style exp on VectorE for a subset of key-block groups (3 blocks
# per group): p = bitcast_bf16(int16(EXP_A*s + EXP_B)) ~ exp(s*SCALE).
# Offloads ~1/3 of softmax work from the bottleneck ScalarE; softmax
# normalization cancels the common-mode bias (measured end-to-end 8e-3).
EXP_A = 128.0 / np.log(2.0) * SCALE
EXP_B = 127.0 * 128.0 - 5.5
DVE_GROUPS = {0: {1}, 1: {1, 3}, 2: set(), 3: set()}


def _tile_perm(x):
    b, s, H, d = x.shape
    x = x.reshape(b, NT, TT, NH, TH, NW, TW, H, d)
    x = x.transpose(0, 1, 3, 5, 2, 4, 6, 7, 8)
    return x.reshape(b, s, H, d)


def _untile_perm(x):
    b, s, H, d = x.shape
    x = x.reshape(b, NT, NH, NW, TT, TH, TW, H, d)
    x = x.transpose(0, 1, 4, 2, 5, 3, 6, 7, 8)
    return x.reshape(b, s, H, d)


def _axis_window(w, n):
    q = np.arange(n)
    start = np.clip(q - (w - 1) // 2, 0, n - w)
    return start[:, None] + np.arange(w)[None, :]


def _tile_gather_idx(wt, wh, ww):
    it = _axis_window(wt, NT)
    ih = _axis_window(wh, NH)
    iw = _axis_window(ww, NW)
    idx = (it[:, None, None, :, None, None] * (NH * NW)
           + ih[None, :, None, None, :, None] * NW
           + iw[None, None, :, None, None, :])
    return idx.reshape(NTILES, wt * wh * ww)


def _npairs(ws):
    return [(3 * w + 1) // 2 for w in ws]     # ceil(3W/2) k-slot pairs per head


# ------------------------------------------------------------- device build
def _split_multi_waits(nc):
    """This container's walrus rejects >1 sem-wait attached to one
    instruction ("Too many sync wait commands"). Hoist extras onto
    single-wait InstNoOps inserted right before, on the same engine."""
    n = 0
    for f in nc.m.functions:
        for b in f.blocks:
            new = []
            for inst in b.instructions:
                si = inst.sync_info
                if si is not None and len(si.on_wait) > 1:
                    for k, w in enumerate(si.on_wait[:-1]):
                        new.append(mybir.InstNoOp(
                            name=f"{inst.name}_wsplit{k}",
                            engine=inst.engine,
                            sync_info=mybir.SyncInfo(on_wait=[w], on_update=[]),
                            bass_nofuse=True,
                        ))
                        n += 1
                    si.on_wait = si.on_wait[-1:]
                new.append(inst)
            b.instructions = new
    return n


def build_kernel(n_qt, ws):
    """One core's program. n_qt qtiles, per-head window sizes ws (list).

    DRAM layouts (per core, bf16 unless noted):
      q:   [128, n_qt*H*384]   col = (j*H + h)*384 + t, d-major; rows 64-127
           duplicate rows 0-63 (for the row-packed second tile_position)
      k:   [128, n_qt*sum(npair)*128]  one [128,128] slot per k-block PAIR:
           rows 0-63 = kT[2p] (d-major [64,128]), rows 64-127 = kT[2p+1]
           col = kphoff[h] + (j*npair_h + p)*128
      v:   [128, n_qt*sum(3*ws)*65]  col = vhoff[h] + (j*3W + kb)*65 + dd
           (token-major [128 keys, 65]; col 64 of each 65-block is ones)
      out: [65, n_qt*H*384] f32  col = (h*n_qt + j)*384 + t
           rows 0..63 = O^T numerator, row 64 = softmax denominator
    """
    H = len(ws)
    nps = _npairs(ws)
    sumnp = sum(nps)
    nc = bass.Bass()
    q_d = nc.declare_dram_parameter("q", [128, n_qt * H * TILE], mybir.dt.bfloat16, isOutput=False)
    k_d = nc.declare_dram_parameter("k", [128, n_qt * sumnp * 128], mybir.dt.bfloat16, isOutput=False)
    v_d = nc.declare_dram_parameter("v", [128, n_qt * 3 * sum(ws) * 65], mybir.dt.bfloat16, isOutput=False)
    o_d = nc.declare_dram_parameter("out", [65, n_qt * H * TILE], mybir.dt.float32, isOutput=True)

    kphoff = np.concatenate([[0], np.cumsum([n_qt * p * 128 for p in nps])])
    vhoff = np.concatenate([[0], np.cumsum([n_qt * 3 * w * 65 for w in ws])])

    with tile.TileContext(nc) as tc:
        with (
            tc.tile_pool(name="qp", bufs=3) as qp,
            tc.tile_pool(name="kp", bufs=4) as kp,
            tc.tile_pool(name="vp", bufs=4) as vp,
            tc.tile_pool(name="pp", bufs=4) as pp,
            tc.tile_pool(name="op", bufs=4) as op,
            tc.tile_pool(name="sp", bufs=2, space="PSUM") as sp,
            tc.tile_pool(name="accp", bufs=2, space="PSUM") as accp,
        ):
            for j in range(n_qt):
                q_sb = qp.tile([128, H * TILE], mybir.dt.bfloat16, tag="q")
                nc.sync.dma_start(q_sb[:], q_d[:, j * H * TILE:(j + 1) * H * TILE])
                for h in range(H):
                    W = ws[h]
                    nkb = 3 * W
                    npair = nps[h]
                    k_sb = kp.tile([128, npair * 128], mybir.dt.bfloat16, tag="k")
                    nc.gpsimd.dma_start(
                        k_sb[:], k_d[:, kphoff[h] + j * npair * 128: kphoff[h] + (j + 1) * npair * 128])
                    v_sb = vp.tile([128, nkb * 65], mybir.dt.bfloat16, tag="v")
                    nc.gpsimd.dma_start(
                        v_sb[:], v_d[:, vhoff[h] + j * nkb * 65: vhoff[h] + (j + 1) * nkb * 65])

                    o_ps = accp.tile([65, TILE], mybir.dt.float32, tag="o")
                    ngr = (nkb + 2) // 3           # exp batches of 3 key-blocks
                    s_tiles = []
                    for g in range(ngr):
                        s_tiles.append(sp.tile([128, 1536], mybir.dt.float32, tag="s",
                                               name=f"s_{j}_{h}_{g}"))
                    for kb in range(nkb):
                        half = kb % 2
                        pr = kb // 2
                        nc.tensor.matmul(
                            s_tiles[kb // 3][:, (kb % 3) * 512:(kb % 3) * 512 + TILE],
                            lhsT=k_sb[64 * half:64 * half + 64, pr * 128:(pr + 1) * 128],
                            rhs=q_sb[64 * half:64 * half + 64, h * TILE:(h + 1) * TILE],
                            start=True, stop=True,
                            tile_position=(64 * half, 0),
                        )
                    for g in range(ngr):
                        nb = min(3, nkb - g * 3)
                        p_sb = pp.tile([128, 3 * TILE], mybir.dt.bfloat16, tag="p",
                                       name=f"p_{j}_{h}_{g}")
                        s_view = s_tiles[g].rearrange("p (b x) -> p b x", x=512)[:, 0:nb, 0:TILE]
                        p_view = p_sb.rearrange("p (b x) -> p b x", x=TILE)[:, 0:nb, :]
                        if g in DVE_GROUPS.get(h, set()):
                            nc.vector.tensor_scalar(
                                p_view.bitcast(mybir.dt.int16), s_view,
                                EXP_A, EXP_B,
                                mybir.AluOpType.mult, mybir.AluOpType.add)
                        else:
                            nc.scalar.activation(
                                p_view, s_view, mybir.ActivationFunctionType.Exp, scale=SCALE)
                        for j3 in range(nb):
                            kb = g * 3 + j3
                            nc.tensor.matmul(
                                o_ps[:],
                                lhsT=v_sb[:, kb * 65:(kb + 1) * 65],
                                rhs=p_sb[:, j3 * TILE:(j3 + 1) * TILE],
                                start=(kb == 0), stop=(kb == nkb - 1),
                            )
                    o_sb = op.tile([65, TILE], mybir.dt.float32, tag="osb")
                    nc.vector.tensor_copy(o_sb[:], o_ps[:])
                    col = (h * n_qt + j) * TILE
                    nc.sync.dma_start(o_d[:, col: col + TILE], o_sb[:])
    _split_multi_waits(nc)
    return nc


# --------------------------------------------------------------- host shard
def shard_inputs(q, k, v, n_cores=N_CORES, qt_per_core=QT_PER_CORE, ws=None):
    """Full [1,S,H,d] f32 inputs -> list of per-core in_maps (bf16 layouts)."""
    if ws is None:
        ws = WS
    H = len(ws)
    nps = _npairs(ws)
    qt_ = _tile_perm(q).reshape(NTILES, TILE, HEADS, D)
    kt_ = _tile_perm(k).reshape(NTILES, TILE, HEADS, D)
    vt_ = _tile_perm(v).reshape(NTILES, TILE, HEADS, D)

    idxs = [_tile_gather_idx(*WINDOWS[h]) for h in range(H)]

    in_maps = []
    for c in range(n_cores):
        sl = slice(c * qt_per_core, (c + 1) * qt_per_core)
        # q: [qt, t, H, d] -> [64, (j h) t], duplicated on rows 64-127
        qc = qt_[sl][:, :, :H, :]                       # [n_qt, 384, H, 64]
        q_half = qc.transpose(3, 0, 2, 1).reshape(D, -1)
        q_arr = np.concatenate([q_half, q_half], axis=0).astype(BF16)
        k_parts, v_parts = [], []
        for h in range(H):
            W = ws[h]
            nkb, npair = 3 * W, nps[h]
            idx = idxs[h][sl]                           # [n_qt, W]
            kg = kt_[idx][:, :, :, h, :]                # [n_qt, W, 384, 64]
            # -> k-blocks [n_qt, nkb, 128, 64], d-major slots [n_qt, npair, 128, 128]
            kgb = kg.reshape(qt_per_core, nkb, 128, D)
            slots = np.zeros((qt_per_core, npair, 128, 128), np.float32)
            for kb in range(nkb):
                # kT d-major [64, 128] into rows 64*(kb%2)..+64 of slot kb//2
                slots[:, kb // 2, 64 * (kb % 2):64 * (kb % 2) + 64, :] = \
                    kgb[:, kb].transpose(0, 2, 1)
            # [n_qt, npair, 128 rows, 128 cols] -> [128, n_qt*npair*128]
            k_parts.append(slots.transpose(2, 0, 1, 3).reshape(128, -1))
            vg = vt_[idx][:, :, :, h, :].reshape(qt_per_core, nkb, 128, D)
            vg1 = np.concatenate([vg, np.ones_like(vg[..., :1])], axis=-1)  # [...,65]
            v_parts.append(vg1.transpose(2, 0, 1, 3).reshape(128, -1))
        k_arr = np.concatenate(k_parts, axis=1).astype(BF16)
        v_arr = np.concatenate(v_parts, axis=1).astype(BF16)
        in_maps.append({"q": np.ascontiguousarray(q_arr),
                        "k": np.ascontiguousarray(k_arr),
                        "v": np.ascontiguousarray(v_arr)})
    return in_maps


def assemble_output(results, n_cores=N_CORES, qt_per_core=QT_PER_CORE):
    """Per-core 'out' [65, H*n_qt*384] f32 -> full [1, S, H, d] f32."""
    parts = []
    for c in range(n_cores):
        arr = results[c]["out"].reshape(65, HEADS, qt_per_core, TILE)
        num = arr[:D]                                   # [64, H, n_qt, 384]
        den = arr[D]                                    # [H, n_qt, 384]
        o = num / den[None]                             # [64, H, n_qt, 384]
        parts.append(o.transpose(2, 3, 1, 0))           # [n_qt, 384, H, 64]
    full = np.concatenate(parts, axis=0)                # [216, 384, H, 64]
    full = full.reshape(1, SEQ, HEADS, D).astype(np.float32)
    return _untile_perm(full)


_BUILT = {}


def kernel(q, k, v):
    key = "full"
    if key not in _BUILT:
        _BUILT[key] = build_kernel(QT_PER_CORE, WS)
    nc = _BUILT[key]
    in_maps = shard_inputs(q, k, v)
    res = run_bass_kernel_spmd(nc, in_maps, core_ids=list(range(N_CORES)))
    return assemble_output(res.results)


# revision 9
# speedup vs baseline: 1.0245x; 1.0245x over previous
"""Sliding-tile attention (STA) for nn_Attention_63548336111668 on 8 TRN2 cores.

Strategy:
- tile()/untile() and the per-head sliding-tile-window gather are pure
  permutations/gathers -> done host-side in numpy.
- Work is (head, query-tile) pairs: 216 qtiles x 4 heads. Each core gets a
  contiguous range of 27 qtiles for ALL 4 heads -> every core runs the exact
  same instruction stream (SPMD) on its own shard.
- Per (h, qt): S^T[k,q] = K_d^T.T @ Q_d^T via TensorE (contraction d=64,
  row-packed in pairs via tile_position so two K=64 matmuls share the
  384-cycle stream), P^T = exp(S^T * 1/8) on ScalarE (scores are O(1), no
  max-subtraction needed), O^T[d,q] += V[k,d].T @ P^T accumulated in PSUM
  over key blocks. V carries an appended ones-column, so O^T row 64 = the
  softmax denominator.
- Host divides numerator rows by the denominator row, then un-permutes.

Compute dtype bf16 (inputs converted host-side), accumulation f32.
"""

import sys

sys.path.insert(0, "/opt/trn_rl_repo")

import numpy as np
import ml_dtypes

import concourse.bass as bass
import concourse.tile as tile
from concourse import mybir
from concourse.bass_utils import run_bass_kernel_spmd

# ---------------------------------------------------------------- geometry
NT, NH, NW = 6, 6, 6
TT, TH, TW = 6, 8, 8
TILE = TT * TH * TW            # 384
NTILES = NT * NH * NW          # 216
SEQ = NTILES * TILE            # 82944
WINDOWS = ((2, 1, 1), (1, 2, 2), (1, 1, 2), (1, 1, 1))
HEADS = 4
D = 64
N_CORES = 8
QT_PER_CORE = NTILES // N_CORES  # 27
SCALE = 1.0 / 8.0              # 1/sqrt(64)
BF16 = ml_dtypes.bfloat16

WS = [w[0] * w[1] * w[2] for w in WINDOWS]   # [2, 4, 2, 1]

# Schraudolph-style exp on VectorE for a subset of key-block groups (3 blocks
# per group): p = bitcast_bf16(int16(EXP_A*s + EXP_B)) ~ exp(s*SCALE).
# Offloads ~1/3 of softmax work from the bottleneck ScalarE; softmax
# normalization cancels the common-mode bias (measured end-to-end 8e-3).
EXP_A = 128.0 / np.log(2.0) * SCALE
EXP_B = 127.0 * 128.0 - 5.5
# groups are now 2 key-blocks each (aligned with the row-packed S-MM pairs)
DVE_GROUPS = {0: {1}, 1: {1, 3, 5}, 2: {1}, 3: set()}
GROUP_KB = 2


def _tile_perm(x):
    b, s, H, d = x.shape
    x = x.reshape(b, NT, TT, NH, TH, NW, TW, H, d)
    x = x.transpose(0, 1, 3, 5, 2, 4, 6, 7, 8)
    return x.reshape(b, s, H, d)


def _untile_perm(x):
    b, s, H, d = x.shape
    x = x.reshape(b, NT, NH, NW, TT, TH, TW, H, d)
    x = x.transpose(0, 1, 4, 2, 5, 3, 6, 7, 8)
    return x.reshape(b, s, H, d)


def _axis_window(w, n):
    q = np.arange(n)
    start = np.clip(q - (w - 1) // 2, 0, n - w)
    return start[:, None] + np.arange(w)[None, :]


def _tile_gather_idx(wt, wh, ww):
    it = _axis_window(wt, NT)
    ih = _axis_window(wh, NH)
    iw = _axis_window(ww, NW)
    idx = (it[:, None, None, :, None, None] * (NH * NW)
           + ih[None, :, None, None, :, None] * NW
           + iw[None, None, :, None, None, :])
    return idx.reshape(NTILES, wt * wh * ww)


def _npairs(ws):
    return [(3 * w + 1) // 2 for w in ws]     # ceil(3W/2) k-slot pairs per head


# ------------------------------------------------------------- device build
def _split_multi_waits(nc):
    """This container's walrus rejects >1 sem-wait attached to one
    instruction ("Too many sync wait commands"). Hoist extras onto
    single-wait InstNoOps inserted right before, on the same engine."""
    n = 0
    for f in nc.m.functions:
        for b in f.blocks:
            new = []
            for inst in b.instructions:
                si = inst.sync_info
                if si is not None and len(si.on_wait) > 1:
                    for k, w in enumerate(si.on_wait[:-1]):
                        new.append(mybir.InstNoOp(
                            name=f"{inst.name}_wsplit{k}",
                            engine=inst.engine,
                            sync_info=mybir.SyncInfo(on_wait=[w], on_update=[]),
                            bass_nofuse=True,
                        ))
                        n += 1
                    si.on_wait = si.on_wait[-1:]
                new.append(inst)
            b.instructions = new
    return n


def build_kernel(n_qt, ws):
    """One core's program. n_qt qtiles, per-head window sizes ws (list).

    DRAM layouts (per core, bf16 unless noted):
      q:   [128, n_qt*H*384]   col = (j*H + h)*384 + t, d-major; rows 64-127
           duplicate rows 0-63 (for the row-packed second tile_position)
      k:   [128, n_qt*sum(npair)*128]  one [128,128] slot per k-block PAIR:
           rows 0-63 = kT[2p] (d-major [64,128]), rows 64-127 = kT[2p+1]
           col = kphoff[h] + (j*npair_h + p)*128
      v:   [128, n_qt*sum(3*ws)*65]  col = vhoff[h] + (j*3W + kb)*65 + dd
           (token-major [128 keys, 65]; col 64 of each 65-block is ones)
      out: [65, n_qt*H*384] f32  col = (h*n_qt + j)*384 + t
           rows 0..63 = O^T numerator, row 64 = softmax denominator
    """
    H = len(ws)
    nps = _npairs(ws)
    sumnp = sum(nps)
    nc = bass.Bass()
    q_d = nc.declare_dram_parameter("q", [128, n_qt * H * TILE], mybir.dt.bfloat16, isOutput=False)
    k_d = nc.declare_dram_parameter("k", [128, n_qt * sumnp * 128], mybir.dt.bfloat16, isOutput=False)
    v_d = nc.declare_dram_parameter("v", [128, n_qt * 3 * sum(ws) * 65], mybir.dt.bfloat16, isOutput=False)
    o_d = nc.declare_dram_parameter("out", [65, n_qt * H * TILE], mybir.dt.float32, isOutput=True)

    kphoff = np.concatenate([[0], np.cumsum([n_qt * p * 128 for p in nps])])
    vhoff = np.concatenate([[0], np.cumsum([n_qt * 3 * w * 65 for w in ws])])

    with tile.TileContext(nc) as tc:
        with (
            tc.tile_pool(name="qp", bufs=3) as qp,
            tc.tile_pool(name="kp", bufs=4) as kp,
            tc.tile_pool(name="vp", bufs=4) as vp,
            tc.tile_pool(name="pp", bufs=4) as pp,
            tc.tile_pool(name="op", bufs=4) as op,
            tc.tile_pool(name="sp", bufs=3, space="PSUM") as sp,
            tc.tile_pool(name="accp", bufs=2, space="PSUM") as accp,
        ):
            for j in range(n_qt):
                q_sb = qp.tile([128, H * TILE], mybir.dt.bfloat16, tag="q")
                nc.sync.dma_start(q_sb[:], q_d[:, j * H * TILE:(j + 1) * H * TILE])
                for h in range(H):
                    W = ws[h]
                    nkb = 3 * W
                    npair = nps[h]
                    k_sb = kp.tile([128, npair * 128], mybir.dt.bfloat16, tag="k")
                    nc.gpsimd.dma_start(
                        k_sb[:], k_d[:, kphoff[h] + j * npair * 128: kphoff[h] + (j + 1) * npair * 128])
                    v_sb = vp.tile([128, nkb * 65], mybir.dt.bfloat16, tag="v")
                    nc.gpsimd.dma_start(
                        v_sb[:], v_d[:, vhoff[h] + j * nkb * 65: vhoff[h] + (j + 1) * nkb * 65])

                    o_ps = accp.tile([65, TILE], mybir.dt.float32, tag="o")
                    GB = GROUP_KB                  # key-blocks per exp batch
                    ngr = (nkb + GB - 1) // GB
                    for g in range(ngr):
                        nb = min(GB, nkb - g * GB)
                        s_ps = sp.tile([128, GB * 512], mybir.dt.float32, tag="s",
                                       name=f"s_{j}_{h}_{g}")
                        for j3 in range(nb):
                            kb = g * GB + j3
                            half = kb % 2
                            pr = kb // 2
                            nc.tensor.matmul(
                                s_ps[:, j3 * 512: j3 * 512 + TILE],
                                lhsT=k_sb[64 * half:64 * half + 64, pr * 128:(pr + 1) * 128],
                                rhs=q_sb[64 * half:64 * half + 64, h * TILE:(h + 1) * TILE],
                                start=True, stop=True,
                                tile_position=(64 * half, 0),
                            )
                        p_sb = pp.tile([128, GB * TILE], mybir.dt.bfloat16, tag="p",
                                       name=f"p_{j}_{h}_{g}")
                        s_view = s_ps.rearrange("p (b x) -> p b x", x=512)[:, 0:nb, 0:TILE]
                        p_view = p_sb.rearrange("p (b x) -> p b x", x=TILE)[:, 0:nb, :]
                        if g in DVE_GROUPS.get(h, set()):
                            nc.vector.tensor_scalar(
                                p_view.bitcast(mybir.dt.int16), s_view,
                                EXP_A, EXP_B,
                                mybir.AluOpType.mult, mybir.AluOpType.add)
                        else:
                            nc.scalar.activation(
                                p_view, s_view, mybir.ActivationFunctionType.Exp, scale=SCALE)
                        for j3 in range(nb):
                            kb = g * GB + j3
                            nc.tensor.matmul(
                                o_ps[:],
                                lhsT=v_sb[:, kb * 65:(kb + 1) * 65],
                                rhs=p_sb[:, j3 * TILE:(j3 + 1) * TILE],
                                start=(kb == 0), stop=(kb == nkb - 1),
                            )
                    o_sb = op.tile([65, TILE], mybir.dt.float32, tag="osb")
                    nc.vector.tensor_copy(o_sb[:], o_ps[:])
                    col = (h * n_qt + j) * TILE
                    nc.sync.dma_start(o_d[:, col: col + TILE], o_sb[:])
    _split_multi_waits(nc)
    return nc


# --------------------------------------------------------------- host shard
def shard_inputs(q, k, v, n_cores=N_CORES, qt_per_core=QT_PER_CORE, ws=None):
    """Full [1,S,H,d] f32 inputs -> list of per-core in_maps (bf16 layouts)."""
    if ws is None:
        ws = WS
    H = len(ws)
    nps = _npairs(ws)
    qt_ = _tile_perm(q).reshape(NTILES, TILE, HEADS, D)
    kt_ = _tile_perm(k).reshape(NTILES, TILE, HEADS, D)
    vt_ = _tile_perm(v).reshape(NTILES, TILE, HEADS, D)

    idxs = [_tile_gather_idx(*WINDOWS[h]) for h in range(H)]

    in_maps = []
    for c in range(n_cores):
        sl = slice(c * qt_per_core, (c + 1) * qt_per_core)
        # q: [qt, t, H, d] -> [64, (j h) t], duplicated on rows 64-127
        qc = qt_[sl][:, :, :H, :]                       # [n_qt, 384, H, 64]
        q_half = qc.transpose(3, 0, 2, 1).reshape(D, -1)
        q_arr = np.concatenate([q_half, q_half], axis=0).astype(BF16)
        k_parts, v_parts = [], []
        for h in range(H):
            W = ws[h]
            nkb, npair = 3 * W, nps[h]
            idx = idxs[h][sl]                           # [n_qt, W]
            kg = kt_[idx][:, :, :, h, :]                # [n_qt, W, 384, 64]
            # -> k-blocks [n_qt, nkb, 128, 64], d-major slots [n_qt, npair, 128, 128]
            kgb = kg.reshape(qt_per_core, nkb, 128, D)
            slots = np.zeros((qt_per_core, npair, 128, 128), np.float32)
            for kb in range(nkb):
                # kT d-major [64, 128] into rows 64*(kb%2)..+64 of slot kb//2
                slots[:, kb // 2, 64 * (kb % 2):64 * (kb % 2) + 64, :] = \
                    kgb[:, kb].transpose(0, 2, 1)
            # [n_qt, npair, 128 rows, 128 cols] -> [128, n_qt*npair*128]
            k_parts.append(slots.transpose(2, 0, 1, 3).reshape(128, -1))
            vg = vt_[idx][:, :, :, h, :].reshape(qt_per_core, nkb, 128, D)
            vg1 = np.concatenate([vg, np.ones_like(vg[..., :1])], axis=-1)  # [...,65]
            v_parts.append(vg1.transpose(2, 0, 1, 3).reshape(128, -1))
        k_arr = np.concatenate(k_parts, axis=1).astype(BF16)
        v_arr = np.concatenate(v_parts, axis=1).astype(BF16)
        in_maps.append({"q": np.ascontiguousarray(q_arr),
                        "k": np.ascontiguousarray(k_arr),
                        "v": np.ascontiguousarray(v_arr)})
    return in_maps


def assemble_output(results, n_cores=N_CORES, qt_per_core=QT_PER_CORE):
    """Per-core 'out' [65, H*n_qt*384] f32 -> full [1, S, H, d] f32."""
    parts = []
    for c in range(n_cores):
        arr = results[c]["out"].reshape(65, HEADS, qt_per_core, TILE)
        num = arr[:D]                                   # [64, H, n_qt, 384]
        den = arr[D]                                    # [H, n_qt, 384]
        o = num / den[None]                             # [64, H, n_qt, 384]
        parts.append(o.transpose(2, 3, 1, 0))           # [n_qt, 384, H, 64]
    full = np.concatenate(parts, axis=0)                # [216, 384, H, 64]
    full = full.reshape(1, SEQ, HEADS, D).astype(np.float32)
    return _untile_perm(full)


_BUILT = {}


def kernel(q, k, v):
    key = "full"
    if key not in _BUILT:
        _BUILT[key] = build_kernel(QT_PER_CORE, WS)
    nc = _BUILT[key]
    in_maps = shard_inputs(q, k, v)
    res = run_bass_kernel_spmd(nc, in_maps, core_ids=list(range(N_CORES)))
    return assemble_output(res.results)


# revision 31
# speedup vs baseline: 1.2423x; 1.2126x over previous
"""Sliding-tile attention (STA) for nn_Attention_63548336111668 on 8 TRN2 cores.

Strategy:
- tile()/untile() and the per-head sliding-tile-window gather are pure
  permutations/gathers -> done host-side in numpy.
- Work is (head, query-tile) pairs: 216 qtiles x 4 heads. Each core gets a
  contiguous range of 27 qtiles for ALL 4 heads -> every core runs the exact
  same instruction stream (SPMD) on its own shard.
- Per (h, qt): S^T[k,q] = K_d^T.T @ Q_d^T via TensorE (contraction d=64,
  row-packed in pairs via tile_position so two K=64 matmuls share the
  384-cycle stream), P^T = exp(S^T * 1/8) on ScalarE (scores are O(1), no
  max-subtraction needed), O^T[d,q] += V[k,d].T @ P^T accumulated in PSUM
  over key blocks. V carries an appended ones-column, so O^T row 64 = the
  softmax denominator.
- Host divides numerator rows by the denominator row, then un-permutes.

Compute dtype bf16 (inputs converted host-side), accumulation f32.
"""

import sys

sys.path.insert(0, "/opt/trn_rl_repo")

import numpy as np
import ml_dtypes

import concourse.bass as bass
import concourse.tile as tile
from concourse import mybir
from concourse.bass_utils import run_bass_kernel_spmd

# ---------------------------------------------------------------- geometry
NT, NH, NW = 6, 6, 6
TT, TH, TW = 6, 8, 8
TILE = TT * TH * TW            # 384
NTILES = NT * NH * NW          # 216
SEQ = NTILES * TILE            # 82944
WINDOWS = ((2, 1, 1), (1, 2, 2), (1, 1, 2), (1, 1, 1))
HEADS = 4
D = 64
N_CORES = 8
QT_PER_CORE = NTILES // N_CORES  # 27
SCALE = 1.0 / 8.0              # 1/sqrt(64)
BF16 = ml_dtypes.bfloat16

WS = [w[0] * w[1] * w[2] for w in WINDOWS]   # [2, 4, 2, 1]

# Schraudolph-style exp on VectorE for a subset of key-block groups (3 blocks
# per group): p = bitcast_bf16(int16(EXP_A*s + EXP_B)) ~ exp(s*SCALE).
# Offloads ~1/3 of softmax work from the bottleneck ScalarE; softmax
# normalization cancels the common-mode bias (measured end-to-end 8e-3).
EXP_A = 128.0 / np.log(2.0) * SCALE
EXP_B = 127.0 * 128.0 - 5.5
# groups are now 2 key-blocks each (aligned with the row-packed S-MM pairs)
DVE_GROUPS = {0: {1}, 1: {1, 3, 5}, 2: {1, 2}, 3: set()}
GROUP_KB = 2


def _tile_perm(x):
    b, s, H, d = x.shape
    x = x.reshape(b, NT, TT, NH, TH, NW, TW, H, d)
    x = x.transpose(0, 1, 3, 5, 2, 4, 6, 7, 8)
    return x.reshape(b, s, H, d)


def _untile_perm(x):
    b, s, H, d = x.shape
    x = x.reshape(b, NT, NH, NW, TT, TH, TW, H, d)
    x = x.transpose(0, 1, 4, 2, 5, 3, 6, 7, 8)
    return x.reshape(b, s, H, d)


def _axis_window(w, n):
    q = np.arange(n)
    start = np.clip(q - (w - 1) // 2, 0, n - w)
    return start[:, None] + np.arange(w)[None, :]


def _tile_gather_idx(wt, wh, ww):
    it = _axis_window(wt, NT)
    ih = _axis_window(wh, NH)
    iw = _axis_window(ww, NW)
    idx = (it[:, None, None, :, None, None] * (NH * NW)
           + ih[None, :, None, None, :, None] * NW
           + iw[None, None, :, None, None, :])
    return idx.reshape(NTILES, wt * wh * ww)


def _npairs(ws):
    return [(3 * w + 1) // 2 for w in ws]     # ceil(3W/2) k-slot pairs per head


# ------------------------------------------------------------- device build
def _split_multi_waits(nc):
    """This container's walrus rejects >1 sem-wait attached to one
    instruction ("Too many sync wait commands"). Hoist extras onto
    single-wait InstNoOps inserted right before, on the same engine."""
    n = 0
    for f in nc.m.functions:
        for b in f.blocks:
            new = []
            for inst in b.instructions:
                si = inst.sync_info
                if si is not None and len(si.on_wait) > 1:
                    for k, w in enumerate(si.on_wait[:-1]):
                        new.append(mybir.InstNoOp(
                            name=f"{inst.name}_wsplit{k}",
                            engine=inst.engine,
                            sync_info=mybir.SyncInfo(on_wait=[w], on_update=[]),
                            bass_nofuse=True,
                        ))
                        n += 1
                    si.on_wait = si.on_wait[-1:]
                new.append(inst)
            b.instructions = new
    return n


def build_kernel(n_qt, ws):
    """One core's program. n_qt qtiles, per-head window sizes ws (list).

    DRAM layouts (per core, bf16 unless noted):
      q:   [128, n_qt*H*384]   col = (j*H + h)*384 + t, d-major; rows 64-127
           duplicate rows 0-63 (for the row-packed second tile_position)
      k:   [128, n_qt*sum(npair)*128]  one [128,128] slot per k-block PAIR:
           rows 0-63 = kT[2p] (d-major [64,128]), rows 64-127 = kT[2p+1]
           col = kphoff[h] + (j*npair_h + p)*128
      v:   [128, n_qt*sum(3*ws)*65]  col = vhoff[h] + (j*3W + kb)*65 + dd
           (token-major [128 keys, 65]; col 64 of each 65-block is ones)
      out: [65, n_qt*H*384] f32  col = (h*n_qt + j)*384 + t
           rows 0..63 = O^T numerator, row 64 = softmax denominator
    """
    H = len(ws)
    nps = _npairs(ws)
    sumnp = sum(nps)
    nc = bass.Bass()
    q_d = nc.declare_dram_parameter("q", [128, n_qt * H * TILE], mybir.dt.bfloat16, isOutput=False)
    k_d = nc.declare_dram_parameter("k", [128, n_qt * sumnp * 128], mybir.dt.bfloat16, isOutput=False)
    v_d = nc.declare_dram_parameter("v", [128, n_qt * 3 * sum(ws) * 65], mybir.dt.bfloat16, isOutput=False)
    o_d = nc.declare_dram_parameter("out", [65, n_qt * H * TILE], mybir.dt.float32, isOutput=True)

    kphoff = np.concatenate([[0], np.cumsum([n_qt * p * 128 for p in nps])])
    vhoff = np.concatenate([[0], np.cumsum([n_qt * 3 * w * 65 for w in ws])])

    with tile.TileContext(nc) as tc:
        with (
            tc.tile_pool(name="qp", bufs=4) as qp,
            tc.tile_pool(name="kp", bufs=8) as kp,
            tc.tile_pool(name="vp", bufs=8) as vp,
            tc.tile_pool(name="pp", bufs=10) as pp,
            tc.tile_pool(name="op", bufs=4) as op,
            tc.tile_pool(name="sp", bufs=3, space="PSUM") as sp,
            tc.tile_pool(name="accp", bufs=2, space="PSUM") as accp,
        ):
            for j in range(n_qt):
                q_sb = qp.tile([128, H * TILE], mybir.dt.bfloat16, tag="q")
                with tc.high_priority(offset=100):
                    nc.sync.dma_start(q_sb[:], q_d[:, j * H * TILE:(j + 1) * H * TILE])
                for h in range(H):
                    W = ws[h]
                    nkb = 3 * W
                    npair = nps[h]
                    k_sb = kp.tile([128, npair * 128], mybir.dt.bfloat16, tag="k")
                    v_sb = vp.tile([128, nkb * 65], mybir.dt.bfloat16, tag="v")
                    with tc.high_priority(offset=100):
                        nc.gpsimd.dma_start(
                            k_sb[:], k_d[:, kphoff[h] + j * npair * 128: kphoff[h] + (j + 1) * npair * 128])
                        nc.sync.dma_start(
                            v_sb[:], v_d[:, vhoff[h] + j * nkb * 65: vhoff[h] + (j + 1) * nkb * 65])

                    o_ps = accp.tile([65, TILE], mybir.dt.float32, tag="o")
                    GB = GROUP_KB                  # key-blocks per exp batch
                    ngr = (nkb + GB - 1) // GB
                    dve_set = DVE_GROUPS.get(h, set())
                    p_tiles = {}
                    for g in range(ngr):
                        nb = min(GB, nkb - g * GB)
                        s_ps = sp.tile([128, GB * 512], mybir.dt.float32, tag="s",
                                       name=f"s_{j}_{h}_{g}")
                        with tc.high_priority(offset=40):
                            for j3 in range(nb):
                                kb = g * GB + j3
                                half = kb % 2
                                pr = kb // 2
                                nc.tensor.matmul(
                                    s_ps[:, j3 * 512: j3 * 512 + TILE],
                                    lhsT=k_sb[64 * half:64 * half + 64, pr * 128:(pr + 1) * 128],
                                    rhs=q_sb[64 * half:64 * half + 64, h * TILE:(h + 1) * TILE],
                                    start=True, stop=True,
                                    tile_position=(64 * half, 0),
                                )
                        p_sb = pp.tile([128, GB * TILE], mybir.dt.bfloat16, tag="p",
                                       name=f"p_{j}_{h}_{g}")
                        p_tiles[g] = p_sb
                        s_view = s_ps.rearrange("p (b x) -> p b x", x=512)[:, 0:nb, 0:TILE]
                        p_view = p_sb.rearrange("p (b x) -> p b x", x=TILE)[:, 0:nb, :]
                        with tc.high_priority(offset=20):
                            if g in dve_set:
                                nc.vector.tensor_scalar(
                                    p_view.bitcast(mybir.dt.int16), s_view,
                                    EXP_A, EXP_B,
                                    mybir.AluOpType.mult, mybir.AluOpType.add)
                            else:
                                nc.scalar.activation(
                                    p_view, s_view, mybir.ActivationFunctionType.Exp, scale=SCALE)
                    # PV chain ordered ACT-groups first, then DVE-groups, to
                    # match expected exp completion order (avoids PE
                    # head-of-line blocking on the accumulation chain)
                    chain = [g for g in range(ngr) if g not in dve_set] + \
                            [g for g in range(ngr) if g in dve_set]
                    ci = 0
                    nchain = nkb
                    for g in chain:
                        nb = min(GB, nkb - g * GB)
                        for j3 in range(nb):
                            kb = g * GB + j3
                            nc.tensor.matmul(
                                o_ps[:],
                                lhsT=v_sb[:, kb * 65:(kb + 1) * 65],
                                rhs=p_tiles[g][:, j3 * TILE:(j3 + 1) * TILE],
                                start=(ci == 0), stop=(ci == nchain - 1),
                            )
                            ci += 1
                    o_sb = op.tile([65, TILE], mybir.dt.float32, tag="osb")
                    with tc.high_priority(offset=30):
                        if h == 2:
                            nc.scalar.copy(o_sb[:], o_ps[:])
                        else:
                            nc.vector.tensor_copy(o_sb[:], o_ps[:])
                        col = (h * n_qt + j) * TILE
                        nc.sync.dma_start(o_d[:, col: col + TILE], o_sb[:])
    _split_multi_waits(nc)
    return nc


# --------------------------------------------------------------- host shard
def shard_inputs(q, k, v, n_cores=N_CORES, qt_per_core=QT_PER_CORE, ws=None):
    """Full [1,S,H,d] f32 inputs -> list of per-core in_maps (bf16 layouts)."""
    if ws is None:
        ws = WS
    H = len(ws)
    nps = _npairs(ws)
    qt_ = _tile_perm(q).reshape(NTILES, TILE, HEADS, D)
    kt_ = _tile_perm(k).reshape(NTILES, TILE, HEADS, D)
    vt_ = _tile_perm(v).reshape(NTILES, TILE, HEADS, D)

    idxs = [_tile_gather_idx(*WINDOWS[h]) for h in range(H)]

    in_maps = []
    for c in range(n_cores):
        sl = slice(c * qt_per_core, (c + 1) * qt_per_core)
        # q: [qt, t, H, d] -> [64, (j h) t], duplicated on rows 64-127
        qc = qt_[sl][:, :, :H, :]                       # [n_qt, 384, H, 64]
        q_half = qc.transpose(3, 0, 2, 1).reshape(D, -1)
        q_arr = np.concatenate([q_half, q_half], axis=0).astype(BF16)
        k_parts, v_parts = [], []
        for h in range(H):
            W = ws[h]
            nkb, npair = 3 * W, nps[h]
            idx = idxs[h][sl]                           # [n_qt, W]
            kg = kt_[idx][:, :, :, h, :]                # [n_qt, W, 384, 64]
            # -> k-blocks [n_qt, nkb, 128, 64], d-major slots [n_qt, npair, 128, 128]
            kgb = kg.reshape(qt_per_core, nkb, 128, D)
            slots = np.zeros((qt_per_core, npair, 128, 128), np.float32)
            for kb in range(nkb):
                # kT d-major [64, 128] into rows 64*(kb%2)..+64 of slot kb//2
                slots[:, kb // 2, 64 * (kb % 2):64 * (kb % 2) + 64, :] = \
                    kgb[:, kb].transpose(0, 2, 1)
            # [n_qt, npair, 128 rows, 128 cols] -> [128, n_qt*npair*128]
            k_parts.append(slots.transpose(2, 0, 1, 3).reshape(128, -1))
            vg = vt_[idx][:, :, :, h, :].reshape(qt_per_core, nkb, 128, D)
            vg1 = np.concatenate([vg, np.ones_like(vg[..., :1])], axis=-1)  # [...,65]
            v_parts.append(vg1.transpose(2, 0, 1, 3).reshape(128, -1))
        k_arr = np.concatenate(k_parts, axis=1).astype(BF16)
        v_arr = np.concatenate(v_parts, axis=1).astype(BF16)
        in_maps.append({"q": np.ascontiguousarray(q_arr),
                        "k": np.ascontiguousarray(k_arr),
                        "v": np.ascontiguousarray(v_arr)})
    return in_maps


def assemble_output(results, n_cores=N_CORES, qt_per_core=QT_PER_CORE):
    """Per-core 'out' [65, H*n_qt*384] f32 -> full [1, S, H, d] f32."""
    parts = []
    for c in range(n_cores):
        arr = results[c]["out"].reshape(65, HEADS, qt_per_core, TILE)
        num = arr[:D]                                   # [64, H, n_qt, 384]
        den = arr[D]                                    # [H, n_qt, 384]
        o = num / den[None]                             # [64, H, n_qt, 384]
        parts.append(o.transpose(2, 3, 1, 0))           # [n_qt, 384, H, 64]
    full = np.concatenate(parts, axis=0)                # [216, 384, H, 64]
    full = full.reshape(1, SEQ, HEADS, D).astype(np.float32)
    return _untile_perm(full)


_BUILT = {}


def kernel(q, k, v):
    q = np.asarray(q, dtype=np.float32)
    k = np.asarray(k, dtype=np.float32)
    v = np.asarray(v, dtype=np.float32)
    key = "full"
    if key not in _BUILT:
        _BUILT[key] = build_kernel(QT_PER_CORE, WS)
    nc = _BUILT[key]
    in_maps = shard_inputs(q, k, v)
    res = run_bass_kernel_spmd(nc, in_maps, core_ids=list(range(N_CORES)))
    return assemble_output(res.results)
